# revision 34
# baseline (speedup 1.0000x reference)
"""LocallyConnected1d Trainium2 kernel.

Problem: out[b, oc, w] = sum_{ic,k} xp[b, ic, w+k] * W[w, oc, ic, k] + bias[oc, w]
  x: (32, 64, 2048) f32, weights: (2048, 64, 64, 3) f32, bias: (64, 2048) f32
  out: (32, 64, 2048) f32.  xp = x padded by 1 on both sides of the last axis.

Sharding: output_width (2048) split into 8 contiguous chunks of 256, one per
NeuronCore.  Weights dominate traffic and are fully sharded this way.

Device data is bf16 (tolerance 2e-2; bf16 end-to-end error ~3e-3).  Per
position w the 193-term contraction (ic x k + bias) is two PSUM-accumulated
matmuls with the X PATCH as the stationary operand (lhsT; 32 columns -> cheap
LDWEIGHTS) and the WEIGHTS as the moving operand (rhs):
  mm1: K=128 rows = (k=0, ic=0..63) ++ (k=1, ic=0..63), lhsT=[128,32b], rhs=[128,64oc]
  mm2: K=65  rows = (k=2, ic=0..63) ++ ones row,        lhsT=[65,32b],  rhs=[65,64oc]

Column-group tiling: position w maps to PE col group j = w%4 via
tile_position=(0, 32j), output to PSUM partitions 32j..32j+32.  LDWEIGHTS for
one col group overlaps MATMULs on the others (per-subarray concurrency),
breaking the serial LDW->MM chain that limits the untiled version (each
LDW ~54ns + MM ~56ns x 512 pairs = 56us serial).  PSUM tiles hold 4 positions
across the full 128 partitions, so the PSUM->SBUF cast copies run at full
partition parallelism.

x is sent ONCE per core as xa[65, OWC+2, B] (ic rows ++ a ones row, padded,
with halo).  mm2's lhsT reads xa columns directly (the ones row doubles as the
bias multiplier).  mm1's stacked [128,*] lhsT is built on-chip by two DVE
copies per slice (k=0 cols ++ k=1 cols).

Host-side prep (numpy -> bf16):
  xa[j, c, b] = xp[b, j, ws+c] for j<64;  xa[64, c, b] = 1.0
  wa[j, c, oc] = W[ws+c, oc, j%64, j//64]   j in [0,128)   (k-major)
  wb[j, c, oc] = W[ws+c, oc, j, 2] for j<64; wb[64, c, oc] = bias[oc, ws+c]
Output out_d[32j+b, t, oc] (bf16) = out[b, oc, ws + 4t + j].
"""

import numpy as np
import ml_dtypes

import concourse.bacc as bacc
import concourse.mybir as mybir
import concourse.tile as tile
from concourse.bass_utils import run_bass_kernel_spmd

B, IC, OC, KS, W = 32, 64, 64, 3, 2048
NCORES = 8
OWC = W // NCORES  # 256 positions per core
QCH = 8            # quads per psum tile: [128, 8, 64] = 2KB f32/part = 1 bank
DCH = 64           # DMA chunk (positions per weight/x fetch)
DT = mybir.dt.bfloat16
F32 = mybir.dt.float32
BF16 = ml_dtypes.bfloat16

_compiled_nc = None


def _build_nc():
    nc = bacc.Bacc("TRN2")

    # xw packs, per (partition j, column c): [wb[j, c, 0:64] | xa[j, c, 0:32]]
    # where xa rows are the padded x window (+ ones row 64) and wb rows are
    # the k=2 weight slice (+ bias row 64).  One tensor -> one DMA per slice.
    xw_d = nc.dram_tensor("xw", [IC + 1, OWC + 2, OC + B], DT, kind="ExternalInput")
    wa_d = nc.dram_tensor("wa", [2 * IC, OWC, OC], DT, kind="ExternalInput")
    out_d = nc.dram_tensor("out", [4 * B, OWC // 4, OC], DT, kind="ExternalOutput")

    # First DMA slice is small so the PE starts quickly; middle slices are
    # fat for descriptor efficiency; the last is small so the final slice's
    # compute tail after the last load is short.
    dma_slices = [(0, 8), (8, 56), (64, 104), (168, 56), (224, 24), (248, 8)]

    with tile.TileContext(nc) as tc:
        with (
            tc.tile_pool(name="w", bufs=3) as wpool,
            tc.tile_pool(name="x", bufs=3) as xpool,
            tc.tile_pool(name="o", bufs=3) as opool,
            tc.tile_pool(name="ps", bufs=4, space="PSUM") as pspool,
        ):
            loaded = []  # (start, len, wa, wb, xa, xb)

            def load_slice(si):
                p0, plen = dma_slices[si]
                sl = slice(p0, p0 + plen)
                slh = slice(p0, p0 + plen + 2)  # +2 halo for x
                wa = wpool.tile([2 * IC, plen, OC], DT, tag="wa", name=f"wa_{si}")
                xw = xpool.tile(
                    [IC + 1, plen + 2, OC + B], DT, tag="xw", name=f"xw_{si}"
                )
                xb = xpool.tile([2 * IC, plen, B], DT, tag="xb", name=f"xb_{si}")
                # split descriptor-generation (DIRECT2D ~600ns fixed per
                # dma_start) across sequencers: sync=wa, scalar=xw.
                # 65-partition transfers stripe over only 13 of 16 DMA queues
                # (ceil(65/16)=5 lines per queue); issuing [0:64] and the last
                # row separately keeps all 16 queues evenly loaded.
                nc.scalar.dma_start(out=xw[0:IC, :, :], in_=xw_d[0:IC, slh, :])
                nc.scalar.dma_start(
                    out=xw[IC : IC + 1, :, :], in_=xw_d[IC : IC + 1, slh, :]
                )
                nc.sync.dma_start(out=wa[:], in_=wa_d[:, sl, :])
                # stack (k=0, k=1) column windows into the 128-row mm1 lhsT
                nc.vector.tensor_copy(
                    out=xb[0:IC, :, :], in_=xw[0:IC, 0:plen, OC : OC + B]
                )
                nc.vector.tensor_copy(
                    out=xb[IC : 2 * IC, :, :], in_=xw[0:IC, 1 : plen + 1, OC : OC + B]
                )
                loaded.append((p0, plen, wa, xw, xb))

            load_slice(0)
            load_slice(1)
            ncopy = 0
            for si in range(len(dma_slices)):
                if si >= 1 and si + 1 < len(dma_slices):
                    load_slice(si + 1)
                p0, plen, wa, xw, xb = loaded[si]
                nq = plen // 4
                ob = opool.tile([4 * B, nq, OC], DT, tag="ob", name=f"ob_{si}")
                for q0 in range(0, nq, QCH):
                    qcl = min(QCH, nq - q0)
                    ps = pspool.tile(
                        [4 * B, qcl, OC], F32, tag="ps", name=f"ps_{p0}_{q0}"
                    )
                    for q in range(qcl):
                        for j in range(4):
                            wl = (q0 + q) * 4 + j
                            nc.tensor.matmul(
                                ps[32 * j : 32 * j + 32, q, :],
                                xb[:, wl, :],
                                wa[:, wl, :],
                                start=True,
                                stop=False,
                                tile_position=(0, 32 * j),
                            )
                        for j in range(4):
                            wl = (q0 + q) * 4 + j
                            nc.tensor.matmul(
                                ps[32 * j : 32 * j + 32, q, :],
                                xw[:, wl + 2, OC : OC + B],
                                xw[:, wl, 0:OC],
                                start=False,
                                stop=True,
                                tile_position=(0, 32 * j),
                            )
                    # PSUM -> SBUF cast copy on DVE (keeps scalar free for
                    # DMA issue and avoids the ACT table load), then ship
                    # this chunk immediately on the gpsimd queue
                    nc.vector.tensor_copy(out=ob[:, q0 : q0 + qcl, :], in_=ps[:])
                    nc.gpsimd.dma_start(
                        out=out_d[:, p0 // 4 + q0 : p0 // 4 + q0 + qcl, :],
                        in_=ob[:, q0 : q0 + qcl, :],
                    )
                    ncopy += 1

    nc.compile()
    return nc


def _get_nc():
    global _compiled_nc
    if _compiled_nc is None:
        _compiled_nc = _build_nc()
    return _compiled_nc


def shard_inputs(x, weights, bias):
    x = np.asarray(x, dtype=np.float32)
    weights = np.asarray(weights, dtype=np.float32)
    bias = np.asarray(bias, dtype=np.float32)

    xp = np.pad(x, ((0, 0), (0, 0), (1, 1)))
    # (IC, W+2, B) in bf16 once, host-side
    xpT = np.ascontiguousarray(xp.transpose(1, 2, 0)).astype(BF16)
    wT = weights.astype(BF16)
    bT = bias.astype(BF16)

    in_maps = []
    for c in range(NCORES):
        ws = c * OWC
        wsl = wT[ws : ws + OWC]  # (OWC, OC, IC, KS)
        wa = np.ascontiguousarray(wsl[:, :, :, 0:2].transpose(3, 2, 0, 1)).reshape(
            2 * IC, OWC, OC
        )
        xw = np.zeros((IC + 1, OWC + 2, OC + B), BF16)
        xw[0:IC, 0:OWC, 0:OC] = wsl[:, :, :, 2].transpose(2, 0, 1)
        xw[IC, 0:OWC, 0:OC] = bT[:, ws : ws + OWC].T
        xw[0:IC, :, OC:] = xpT[:, ws : ws + OWC + 2, :]
        xw[IC, :, OC:] = 1.0
        in_maps.append(
            {
                "xw": np.ascontiguousarray(xw),
                "wa": np.ascontiguousarray(wa),
            }
        )
    return in_maps


def run_sharded(x, weights, bias, trace=False):
    nc = _get_nc()
    in_maps = shard_inputs(x, weights, bias)
    res = run_bass_kernel_spmd(nc, in_maps, list(range(NCORES)), trace=trace)
    out = np.empty((B, OC, W), np.float32)
    for c in range(NCORES):
        ws = c * OWC
        # res [4j*32+b, t, oc] -> out[b, oc, ws + 4t + j]
        arr = res.results[c]["out"].astype(np.float32)
        arr = arr.reshape(4, B, OWC // 4, OC).transpose(1, 3, 2, 0)  # b, oc, t, j
        out[:, :, ws : ws + OWC] = arr.reshape(B, OC, OWC)
    return out, res


def kernel(x, weights, bias):
    out, _ = run_sharded(x, weights, bias)
    return out
